# revision 1
# baseline (speedup 1.0000x reference)
"""Trainium2 Bass kernel for an attention-decoder LSTM (nn_Decoder).

Data-parallel over 8 NeuronCores: batch 4096 -> 512 per core. All weights
replicated. The T-1=127 step recurrence runs fully on-chip: enc_proj is
precomputed once into SBUF (bf16, [ENC, T, B] layout) and every step does
  hp   = 0.5*W1_h.T @ H + 0.5*W1_c.T @ C          (PE, H=2h, C=2c)
  X    = tanh(enc_proj + hp)                       (DVE add + ACT tanh)
  e    = w2.T @ X      -> PSUM rows [t, b]         (PE, M=1, row offset t)
  S    = exp(e)                                    (ACT)
  den  = ones.T @ S ; num = ones.T @ (S*pfc)       (PE)
  r    = num / den                                 (DVE reciprocal + mult)
  gates= 0.5*W_hh.T @ H + W_ih*r + fc_wy*W_ih*y    (PE; fc_b folded in bias)
  LSTM update via tanh-only form (no division, no sigmoid table)
Final output row: 0.5*Wfh.T @ H + (ones.T @ (S*pfin))/den + fc_final_b.
"""

import numpy as np
import ml_dtypes

import concourse.bass as bass
import concourse.bacc as bacc
import concourse.tile as tile
from concourse import mybir
from concourse.bass_utils import run_bass_kernel_spmd

NCORES = 8
B_FULL, T, E, D = 4096, 128, 128, 128
B = B_FULL // NCORES        # 512 batch per core
TSTEPS = T - 1              # 127
TC = 8                      # t-chunk for the big add/tanh passes
NBLK = B // 128             # 4 b-blocks of 128 for input transpose

FP = mybir.dt.float32
BF = mybir.dt.bfloat16
AF = mybir.ActivationFunctionType
OP = mybir.AluOpType
BF_NP = ml_dtypes.bfloat16


def _build(fc_wy: float, fc_final_b: float, n_steps: int):
    nc = bacc.Bacc("TRN2", target_bir_lowering=False, debug=False,
                   num_devices=NCORES)

    x_ext = nc.declare_dram_parameter("x", [B, T, E], FP, isOutput=False)
    yh_ext = nc.declare_dram_parameter("yh", [TSTEPS, B], BF, isOutput=False)
    # [0.5*W1_h.T | 0.5*W1_c.T]  -> [D, 2E]
    w1hc_ext = nc.declare_dram_parameter("w1hc", [D, 2 * E], BF, isOutput=False)
    wke_ext = nc.declare_dram_parameter("wke", [E, E], BF, isOutput=False)  # W1_e.T
    # shifted one-hot stationaries: zeros except column 127 = vec
    w2g_ext = nc.declare_dram_parameter("w2g", [E, 2 * T], BF, isOutput=False)
    gfc_ext = nc.declare_dram_parameter("gfc", [E, 2 * T], BF, isOutput=False)
    gfin_ext = nc.declare_dram_parameter("gfin", [E, 2 * T], BF, isOutput=False)
    whh_ext = nc.declare_dram_parameter("whh", [D, 4 * D], BF, isOutput=False)  # 0.5*W_hh.T
    wih_ext = nc.declare_dram_parameter("wih", [1, 4 * D], BF, isOutput=False)  # W_ih col
    gb_ext = nc.declare_dram_parameter("gb", [D, 4], FP, isOutput=False)
    b1_ext = nc.declare_dram_parameter("b1", [E, 1], FP, isOutput=False)
    wfh_ext = nc.declare_dram_parameter("wfh", [D, 1], BF, isOutput=False)  # 0.5*Wfh
    id_ext = nc.declare_dram_parameter("ident", [128, 128], BF, isOutput=False)
    out_ext = nc.declare_dram_parameter("out", [1, B], FP, isOutput=True)

    with tile.TileContext(nc) as tc:
        import contextlib
        _stack = contextlib.ExitStack()
        const = _stack.enter_context(tc.tile_pool(name="const", bufs=1))
        work = _stack.enter_context(tc.tile_pool(name="work", bufs=2))
        work1 = _stack.enter_context(tc.tile_pool(name="work1", bufs=1))
        dma4 = _stack.enter_context(tc.tile_pool(name="dma4", bufs=4))
        ps1 = _stack.enter_context(tc.tile_pool(name="ps1", bufs=4, space="PSUM"))
        ps2 = _stack.enter_context(tc.tile_pool(name="ps2", bufs=2, space="PSUM"))
        ps3 = _stack.enter_context(tc.tile_pool(name="ps3", bufs=2, space="PSUM"))

        # ---- constants -------------------------------------------------
        w1hc_sb = const.tile([D, 2 * E], BF, tag="w1hc")
        nc.sync.dma_start(out=w1hc_sb[:], in_=w1hc_ext[:])
        wke_sb = const.tile([E, E], BF, tag="wke")
        nc.sync.dma_start(out=wke_sb[:], in_=wke_ext[:])
        w2g_sb = const.tile([E, 2 * T], BF, tag="w2g")
        nc.sync.dma_start(out=w2g_sb[:], in_=w2g_ext[:])
        gfc_sb = const.tile([E, 2 * T], BF, tag="gfc")
        nc.sync.dma_start(out=gfc_sb[:], in_=gfc_ext[:])
        gfin_sb = const.tile([E, 2 * T], BF, tag="gfin")
        nc.sync.dma_start(out=gfin_sb[:], in_=gfin_ext[:])
        whh_sb = const.tile([D, 4 * D], BF, tag="whh")
        nc.sync.dma_start(out=whh_sb[:], in_=whh_ext[:])
        wih_sb = const.tile([1, 4 * D], BF, tag="wih")
        nc.sync.dma_start(out=wih_sb[:], in_=wih_ext[:])
        gb_sb = const.tile([D, 4], FP, tag="gb")
        nc.sync.dma_start(out=gb_sb[:], in_=gb_ext[:])
        b1_sb = const.tile([E, 1], FP, tag="b1")
        nc.sync.dma_start(out=b1_sb[:], in_=b1_ext[:])
        wfh_sb = const.tile([D, 1], BF, tag="wfh")
        nc.sync.dma_start(out=wfh_sb[:], in_=wfh_ext[:])
        id_sb = const.tile([128, 128], BF, tag="ident")
        nc.sync.dma_start(out=id_sb[:], in_=id_ext[:])
        ones_sb = const.tile([T, 1], BF, tag="ones")
        nc.vector.memset(ones_sb[:], 1.0)

        encp = const.tile([E, T, B], BF, tag="encp")
        pfc_sb = const.tile([T, B], BF, tag="pfc")
        pfin_sb = const.tile([T, B], BF, tag="pfin")
        H = const.tile([D, B], FP, tag="H")   # 2*h
        C = const.tile([D, B], FP, tag="C")   # 2*c
        nc.vector.memset(H[:], 0.0)
        nc.vector.memset(C[:], 0.0)

        # ---- precompute: enc_proj, pfc, pfin ---------------------------
        pfc_ps = ps2.tile([T, B], FP, tag="p2")
        pfin_ps = ps2.tile([T, B], FP, tag="p2")
        for t in range(T):
            inT_ps = ps1.tile([E, B], BF, tag="big")
            for blk in range(NBLK):
                xin = dma4.tile([128, E], FP, tag="xin")
                nc.sync.dma_start(
                    out=xin[:],
                    in_=x_ext[blk * 128:(blk + 1) * 128, t, :],
                )
                xbf = work1.tile([128, E], BF, tag="xbf")
                nc.vector.tensor_copy(xbf[:], xin[:])
                nc.tensor.transpose(
                    inT_ps[:, blk * 128:(blk + 1) * 128], xbf[:], id_sb[:]
                )
            inT = work.tile([E, B], BF, tag="inT")
            nc.vector.tensor_copy(inT[:], inT_ps[:])
            ep_ps = ps1.tile([E, B], FP, tag="big")
            nc.tensor.matmul(ep_ps[:], wke_sb[:], inT[:],
                             start=True, stop=True)
            nc.tensor.matmul(pfc_ps[:], gfc_sb[:, T - 1 - t:2 * T - 1 - t],
                             inT[:], start=(t == 0), stop=(t == T - 1))
            nc.tensor.matmul(pfin_ps[:], gfin_sb[:, T - 1 - t:2 * T - 1 - t],
                             inT[:], start=(t == 0), stop=(t == T - 1))
            # enc_proj + attn_b1, cast to bf16, store [E, t, B]
            nc.scalar.activation(encp[:, t, :], ep_ps[:],
                                 AF.Identity, bias=b1_sb[:], scale=1.0)
        nc.vector.tensor_copy(pfc_sb[:], pfc_ps[:])
        nc.vector.tensor_copy(pfin_sb[:], pfin_ps[:])

        # initial bf16 state casts (zeros)
        Hbf = work.tile([D, B], BF, tag="Hbf")
        Cbf = work.tile([D, B], BF, tag="Cbf")
        nc.vector.memset(Hbf[:], 0.0)
        nc.vector.memset(Cbf[:], 0.0)

        rcp = None
        S_sb = None

        # ---- the recurrence -------------------------------------------
        for s in range(n_steps):
            yrow = dma4.tile([1, B], BF, tag="yrow")
            nc.sync.dma_start(out=yrow[:], in_=yh_ext[s:s + 1, :])
            # hp = 0.5*W1h.T @ H + 0.5*W1c.T @ C   [E, B]
            hp_ps = ps3.tile([E, B], FP, tag="hp")
            nc.tensor.matmul(hp_ps[:], w1hc_sb[:, 0:E], Hbf[:],
                             start=True, stop=False)
            nc.tensor.matmul(hp_ps[:], w1hc_sb[:, E:2 * E], Cbf[:],
                             start=False, stop=True)
            hp_sb = work.tile([E, B], BF, tag="hp_sb")
            nc.vector.tensor_copy(hp_sb[:], hp_ps[:])
            hp_b = hp_sb[:].unsqueeze(1).broadcast_to([E, TC, B])

            e_ps = ps1.tile([T, B], FP, tag="big")
            for tcid in range(T // TC):
                X = work.tile([E, TC, B], BF, tag="X")
                nc.vector.tensor_tensor(
                    X[:], encp[:, tcid * TC:(tcid + 1) * TC, :], hp_b, op=OP.add
                )
                nc.scalar.activation(X[:], X[:], AF.Tanh)
                for j in range(TC):
                    t = tcid * TC + j
                    nc.tensor.matmul(e_ps[:], w2g_sb[:, T - 1 - t:2 * T - 1 - t],
                                     X[:, j, :], start=(t == 0), stop=(t == T - 1))

            S_sb = work1.tile([T, B], BF, tag="S")
            nc.scalar.activation(S_sb[:], e_ps[:], AF.Exp)
            SP = work1.tile([T, B], BF, tag="SP")
            nc.vector.tensor_tensor(SP[:], S_sb[:], pfc_sb[:], op=OP.mult)

            den_ps = ps2.tile([1, B], FP, tag="p2")
            nc.tensor.matmul(den_ps[:], ones_sb[:], S_sb[:],
                             start=True, stop=True)
            num_ps = ps2.tile([1, B], FP, tag="p2")
            nc.tensor.matmul(num_ps[:], ones_sb[:], SP[:],
                             start=True, stop=True)

            rcp = work1.tile([1, B], FP, tag="rcp")
            nc.vector.reciprocal(rcp[:], den_ps[:])
            r = work1.tile([1, B], FP, tag="r")
            nc.vector.tensor_tensor(r[:], num_ps[:], rcp[:], op=OP.mult)
            # y_tilde (sans fc_b, folded into gate bias) as bf16 row
            yt = work1.tile([1, B], BF, tag="yt")
            nc.vector.scalar_tensor_tensor(yt[:], yrow[:], fc_wy, r[:],
                                           op0=OP.mult, op1=OP.add)

            # gates: g = 0.5*Whh.T @ H + W_ih (x) y_tilde
            tg = []
            for g in range(4):
                g_ps = ps1.tile([D, B], FP, tag="big")
                nc.tensor.matmul(g_ps[:], whh_sb[:, g * D:(g + 1) * D], Hbf[:],
                                 start=True, stop=False)
                nc.tensor.matmul(g_ps[:], wih_sb[:, g * D:(g + 1) * D], yt[:],
                                 start=False, stop=True)
                tgt = work1.tile([D, B], FP, tag=f"tg{g}")
                scale = 1.0 if g == 2 else 0.5
                nc.scalar.activation(tgt[:], g_ps[:], AF.Tanh,
                                     bias=gb_sb[:, g:g + 1], scale=scale)
                tg.append(tgt)

            # C_new(=2c) = 0.5*(tf+1)*C + (ti+1)*tg ; H_new(=2h) = (to+1)*tanh(c)
            tmp1 = work1.tile([D, B], FP, tag="tmp1")
            nc.vector.scalar_tensor_tensor(tmp1[:], tg[1][:], 1.0, C[:],
                                           op0=OP.add, op1=OP.mult)
            tmp2 = work1.tile([D, B], FP, tag="tmp2")
            nc.vector.scalar_tensor_tensor(tmp2[:], tg[0][:], 1.0, tg[2][:],
                                           op0=OP.add, op1=OP.mult)
            nc.vector.scalar_tensor_tensor(C[:], tmp1[:], 0.5, tmp2[:],
                                           op0=OP.mult, op1=OP.add)
            tct = work1.tile([D, B], FP, tag="tct")
            nc.scalar.activation(tct[:], C[:], AF.Tanh, scale=0.5)
            nc.vector.scalar_tensor_tensor(H[:], tg[3][:], 1.0, tct[:],
                                           op0=OP.add, op1=OP.mult)
            Hbf = work.tile([D, B], BF, tag="Hbf")
            nc.vector.tensor_copy(Hbf[:], H[:])
            Cbf = work.tile([D, B], BF, tag="Cbf")
            nc.vector.tensor_copy(Cbf[:], C[:])

        # ---- final output row ----------------------------------------
        o_ps = ps2.tile([1, B], FP, tag="p2")
        nc.tensor.matmul(o_ps[:], wfh_sb[:], Hbf[:], start=True, stop=True)
        if n_steps > 0:
            SPf = work1.tile([T, B], BF, tag="SP")
            nc.vector.tensor_tensor(SPf[:], S_sb[:], pfin_sb[:], op=OP.mult)
            nf_ps = ps2.tile([1, B], FP, tag="p2")
            nc.tensor.matmul(nf_ps[:], ones_sb[:], SPf[:], start=True, stop=True)
            rfin = work1.tile([1, B], FP, tag="r")
            nc.vector.tensor_tensor(rfin[:], nf_ps[:], rcp[:], op=OP.mult)
            o_sb = work1.tile([1, B], FP, tag="osb")
            nc.vector.scalar_tensor_tensor(o_sb[:], o_ps[:], fc_final_b, rfin[:],
                                           op0=OP.add, op1=OP.add)
        else:
            o_sb = work1.tile([1, B], FP, tag="osb")
            nc.vector.tensor_scalar_add(o_sb[:], o_ps[:], fc_final_b)
        nc.sync.dma_start(out=out_ext[:], in_=o_sb[:])
        _stack.close()

    nc.finalize()
    return nc


def _prep_host(inputs, n_steps):
    f32 = np.float32
    attn_W1 = np.asarray(inputs["attn_W1"], f32)
    attn_W2 = np.asarray(inputs["attn_W2"], f32)
    W_ih = np.asarray(inputs["W_ih"], f32)
    W_hh = np.asarray(inputs["W_hh"], f32)
    b_ih = np.asarray(inputs["b_ih"], f32)
    b_hh = np.asarray(inputs["b_hh"], f32)
    fc_W = np.asarray(inputs["fc_W"], f32)
    fc_b = np.asarray(inputs["fc_b"], f32)
    fcf_W = np.asarray(inputs["fc_final_W"], f32)
    fcf_b = np.asarray(inputs["fc_final_b"], f32)

    W1_h = attn_W1[:, :D]
    W1_c = attn_W1[:, D:2 * D]
    W1_e = attn_W1[:, 2 * D:]

    w1hc = np.concatenate([0.5 * W1_h.T, 0.5 * W1_c.T], axis=1)      # [D, 2E]
    wke = np.ascontiguousarray(W1_e.T)                                # [E, E]
    def onehot_shift(vec):
        g = np.zeros((E, 2 * T), f32)
        g[:, T - 1] = vec
        return g.astype(BF_NP)
    w2g = onehot_shift(attn_W2[0])
    gfc = onehot_shift(fc_W[0, :E])
    gfin = onehot_shift(fcf_W[0, D:])
    whh = 0.5 * W_hh.T                                                # [D, 4D]
    wih = W_ih[:, 0][None, :]                                         # [1, 4D]
    fc_wy = float(fc_W[0, E])
    wfh = 0.5 * fcf_W[0, :D][:, None]                                 # [D, 1]

    bs = b_ih + b_hh + W_ih[:, 0] * float(fc_b[0])                    # [4D]
    scales = np.array([0.5, 0.5, 1.0, 0.5], f32)
    gb = np.stack([bs[g * D:(g + 1) * D] * scales[g] for g in range(4)],
                  axis=1)                                             # [D, 4]
    b1 = np.asarray(inputs["attn_b1"], f32)[:, None]

    weights = {
        "w1hc": w1hc.astype(BF_NP), "wke": wke.astype(BF_NP),
        "w2g": w2g, "gfc": gfc, "gfin": gfin, "whh": whh.astype(BF_NP),
        "wih": wih.astype(BF_NP),
        "gb": gb.astype(f32), "b1": b1.astype(f32),
        "wfh": wfh.astype(BF_NP),
        "ident": np.eye(128, dtype=f32).astype(BF_NP),
    }

    x_full = np.ascontiguousarray(np.asarray(inputs["input_encoded"], f32))
    yh_full = np.asarray(inputs["y_history"], f32)[:, :, 0]           # [B_FULL, 127]

    in_maps = []
    for i in range(NCORES):
        sl = slice(i * B, (i + 1) * B)
        m = dict(weights)
        m["x"] = x_full[sl]
        m["yh"] = np.ascontiguousarray(yh_full[sl].T).astype(BF_NP)   # [127, B]
        in_maps.append(m)
    return in_maps, fc_wy, float(fcf_b[0])


_RUN_KW = {}


def _kernel_impl(inputs, n_steps):
    in_maps, fc_wy, fcf_b = _prep_host(inputs, n_steps)
    nc = _build(fc_wy, fcf_b, n_steps)
    res = run_bass_kernel_spmd(nc, in_maps, core_ids=list(range(NCORES)),
                               **_RUN_KW)
    out = np.concatenate(
        [np.asarray(res.results[i]["out"], np.float32).reshape(B, 1)
         for i in range(NCORES)], axis=0)
    return out, res


def kernel(**inputs) -> np.ndarray:
    out, _ = _kernel_impl(inputs, TSTEPS)
    return out

